# revision 1
# baseline (speedup 1.0000x reference)
"""MoE block (8 experts, top-2, shared SwiGLU expert) on 8 TRN2 NeuronCores.

Strategy (expert-parallel):
  - Core e owns expert e: computes c_e(token) * silu(x @ w1[e]) @ w2[e] for ALL
    tokens (dense, combine weight c_e is zero for non-selected tokens).
  - Shared expert is tensor-parallel over its inter dim F: core e owns a 128-wide
    slice of gate/up columns and the matching shared_down rows; the sigmoid token
    gate is folded into the slice contribution before the down matmul.
  - Router (+ shared gate logit as a 9th column) is replicated on every core.
  - Per-core partial y^T [D, N] accumulates routed + shared-slice contributions;
    a ReduceScatter(add) over the 8 cores both reduces and shards the result.
    Core r returns rows [128*r, 128*(r+1)) of the final y^T; the host
    concatenates and transposes.

All matmuls run in float32r (fp32 transpose-mode streaming: 1 cycle/row when the
moving free dim >= 256) with the data shipped pre-transposed (x^T) so every
operand is already in lhsT layout.
"""

import numpy as np

D = 1024
F = 1024
E = 8
B, T = 2, 2048
N = B * T          # 4096 tokens
NCORES = 8
CHUNK = 512        # tokens per pipeline chunk
NCHUNK = N // CHUNK
FSL = F // NCORES  # shared-expert inter-dim slice per core
DSH = D // NCORES  # output rows (of y^T) per core after reduce-scatter

_CACHE = {}


def _build_nc():
    import concourse.bacc as bacc
    import concourse.mybir as mybir
    import concourse.tile as tile
    from concourse import masks

    dt = mybir.dt
    f32 = dt.float32
    f32r = dt.float32r
    Act = mybir.ActivationFunctionType
    Alu = mybir.AluOpType
    AX = mybir.AxisListType

    nc = bacc.Bacc(
        "TRN2",
        target_bir_lowering=False,
        debug=False,
        enable_asserts=False,
        num_devices=NCORES,
    )

    xT = nc.dram_tensor("xT", [D, N], f32, kind="ExternalInput").ap()
    rw9 = nc.dram_tensor("rw9", [D, E + 1], f32, kind="ExternalInput").ap()
    w1 = nc.dram_tensor("w1", [D, F], f32, kind="ExternalInput").ap()
    w2 = nc.dram_tensor("w2", [F, D], f32, kind="ExternalInput").ap()
    sg = nc.dram_tensor("sg", [D, FSL], f32, kind="ExternalInput").ap()
    su = nc.dram_tensor("su", [D, FSL], f32, kind="ExternalInput").ap()
    sd = nc.dram_tensor("sd", [FSL, D], f32, kind="ExternalInput").ap()
    esel = nc.dram_tensor("esel", [1, E], f32, kind="ExternalInput").ap()
    out = nc.dram_tensor("out", [DSH, N], f32, kind="ExternalOutput").ap()

    r = lambda ap: ap.bitcast(f32r)

    with tile.TileContext(nc) as tc:
        with (
            tc.tile_pool(name="wp", bufs=1) as wp,
            tc.tile_pool(name="xp", bufs=2) as xp,
            tc.tile_pool(name="sp", bufs=2) as sp,
            tc.tile_pool(name="vp", bufs=2) as vp,
            tc.tile_pool(name="pp", bufs=1, space="PSUM") as pp,
            tc.tile_pool(name="dp", bufs=2, space="DRAM") as dp,
        ):
            # ---- static weights/constants ----
            # f32r matmul operands must be produced as f32r by a compute op,
            # so weights are staged f32 then cast once on DVE.
            w1_t = []
            w2_t = []
            sg_t = []
            su_t = []
            rw_t = []
            with tc.tile_pool(name="stg", bufs=2) as stg:
                def load_r(dst_pool, tag, src_ap, nparts, nfree):
                    st = stg.tile([nparts, nfree], f32, tag="stage", name="st")
                    nc.sync.dma_start(st[:], src_ap)
                    t = dst_pool.tile([nparts, nfree], f32r, tag=tag, name=tag)
                    nc.vector.tensor_copy(t[:], st[:])
                    return t

                for db in range(8):
                    w1_t.append(load_r(wp, f"w1_{db}", w1[db * 128:(db + 1) * 128, :], 128, F))
                for fb in range(8):
                    w2_t.append(load_r(wp, f"w2_{fb}", w2[fb * 128:(fb + 1) * 128, :], 128, D))
                for db in range(8):
                    sg_t.append(load_r(wp, f"sg_{db}", sg[db * 128:(db + 1) * 128, :], 128, FSL))
                    su_t.append(load_r(wp, f"su_{db}", su[db * 128:(db + 1) * 128, :], 128, FSL))
                sd_t = load_r(wp, "sd", sd[:, :], 128, D)
            for db in range(8):
                t = wp.tile([128, E + 1], f32, tag=f"rw_{db}", name="rwt")
                nc.sync.dma_start(t[:], rw9[db * 128:(db + 1) * 128, :])
                rw_t.append(t)
            esel_sb = wp.tile([1, E], f32, tag="esel1")
            nc.sync.dma_start(esel_sb[:], esel[:, :])
            esel_bc = wp.tile([128, E], f32, tag="eselbc")
            nc.gpsimd.partition_broadcast(esel_bc[:], esel_sb[:])
            ident = wp.tile([128, 128], f32, tag="ident")
            masks.make_identity(nc, ident[:])

            # ---- main pipeline over token chunks ----
            for c in range(NCHUNK):
                tok0 = c * CHUNK
                # x^T chunk, 8 partition blocks of [128, CHUNK]
                xcf = []
                xc = []
                for db in range(8):
                    tf_ = xp.tile([128, CHUNK], f32, tag=f"xcf{db}", bufs=1, name="tf_")
                    nc.sync.dma_start(
                        tf_[:], xT[db * 128:(db + 1) * 128, tok0:tok0 + CHUNK]
                    )
                    xcf.append(tf_)
                    tr_ = xp.tile([128, CHUNK], f32r, tag=f"xc{db}", name="tr_")
                    nc.vector.tensor_copy(tr_[:], tf_[:])
                    xc.append(tr_)

                # --- router + shared gate logit, token-partition layout ---
                cT = vp.tile([1, CHUNK], f32, tag="cT")
                gT = vp.tile([1, CHUNK], f32, tag="gT")
                for s in range(CHUNK // 128):
                    lg_ps = pp.tile([128, E + 1], f32, tag="lg")
                    for db in range(8):
                        nc.tensor.matmul(
                            lg_ps[:],
                            lhsT=xcf[db][:, s * 128:(s + 1) * 128],
                            rhs=rw_t[db][:],
                            start=(db == 0),
                            stop=(db == 7),
                        )
                    lg = vp.tile([128, E], f32, tag="lg_sb")
                    nc.vector.tensor_copy(lg[:], lg_ps[:, 0:E])
                    # softmax numerator + denominator (no max-subtract: |logit|<~6)
                    pe_un = vp.tile([128, E], f32, tag="pe_un")
                    sumx = vp.tile([128, 1], f32, tag="sumx")
                    nc.scalar.activation(
                        pe_un[:], lg_ps[:, 0:E], Act.Exp, accum_out=sumx[:]
                    )
                    rcp = vp.tile([128, 1], f32, tag="rcp")
                    nc.vector.reciprocal(rcp[:], sumx[:])
                    # rank_i = #{j: l_j > l_i}  (strict; top-2 keep rank < 2)
                    cnt = [
                        vp.tile([128, E], f32, tag="cnt0", name="cnt0"),
                        vp.tile([128, E], f32, tag="cnt1", name="cnt1"),
                    ]
                    nc.vector.tensor_scalar(
                        cnt[0][:], lg[:], lg[:, 0:1], None, Alu.is_lt
                    )
                    for j in range(1, E):
                        nc.vector.scalar_tensor_tensor(
                            cnt[j % 2][:],
                            lg[:],
                            lg[:, j:j + 1],
                            cnt[(j + 1) % 2][:],
                            Alu.is_lt,
                            Alu.add,
                        )
                    cfin = cnt[(E - 1) % 2]
                    mask = vp.tile([128, E], f32, tag="mask")
                    nc.vector.tensor_scalar(
                        mask[:], cfin[:], 2.0, None, Alu.is_lt
                    )
                    t1 = vp.tile([128, E], f32, tag="t1")
                    nc.vector.tensor_mul(t1[:], pe_un[:], mask[:])
                    t2 = vp.tile([128, E], f32, tag="t2")
                    nc.vector.tensor_mul(t2[:], t1[:], esel_bc[:])
                    cred = vp.tile([128, 1], f32, tag="cred")
                    nc.vector.reduce_sum(cred[:], t2[:], axis=AX.X)
                    ccol = vp.tile([128, 1], f32, tag="ccol")
                    nc.vector.tensor_scalar_mul(ccol[:], cred[:], rcp[:])
                    sig = vp.tile([128, 1], f32, tag="sig")
                    nc.scalar.activation(sig[:], lg_ps[:, E:E + 1], Act.Sigmoid)
                    # transpose both [128,1] columns into row layout
                    ct_ps = pp.tile([1, 256], f32, tag="ct")
                    nc.tensor.transpose(ct_ps[:, 0:128], ccol[:], ident[:])
                    nc.tensor.transpose(ct_ps[:, 128:256], sig[:], ident[:])
                    nc.vector.tensor_copy(
                        cT[:, s * 128:(s + 1) * 128], ct_ps[:, 0:128]
                    )
                    nc.vector.tensor_copy(
                        gT[:, s * 128:(s + 1) * 128], ct_ps[:, 128:256]
                    )
                bc_c = sp.tile([128, CHUNK], f32, tag="bc_c")
                nc.gpsimd.partition_broadcast(bc_c[:], cT[:])
                bc_g = sp.tile([128, CHUNK], f32, tag="bc_g")
                nc.gpsimd.partition_broadcast(bc_g[:], gT[:])

                # --- expert up-proj + silu + combine scale ---
                hp = []
                for fb in range(8):
                    h_ps = pp.tile([128, CHUNK], f32, tag="h", bufs=2)
                    for db in range(8):
                        nc.tensor.matmul(
                            h_ps[:],
                            lhsT=w1_t[db][:, fb * 128:(fb + 1) * 128],
                            rhs=xc[db][:],
                            start=(db == 0),
                            stop=(db == 7),
                        )
                    h_sg = sp.tile([128, CHUNK], f32, tag="tmp", bufs=6, name="h_sg")
                    nc.scalar.activation(h_sg[:], h_ps[:], Act.Sigmoid)
                    h_s = sp.tile([128, CHUNK], f32, tag="tmp", bufs=6, name="h_s")
                    nc.vector.tensor_mul(h_s[:], h_sg[:], h_ps[:])
                    hpt = sp.tile([128, CHUNK], f32r, tag=f"hp{fb}", name="hpt")
                    nc.vector.tensor_mul(hpt[:], h_s[:], bc_c[:])
                    hp.append(hpt)

                # --- shared expert slice: silu(gate)*up*sigmoid ---
                g_ps = pp.tile([128, CHUNK], f32, tag="g")
                u_ps = pp.tile([128, CHUNK], f32, tag="u")
                for db in range(8):
                    nc.tensor.matmul(
                        g_ps[:],
                        lhsT=sg_t[db][:],
                        rhs=xc[db][:],
                        start=(db == 0),
                        stop=(db == 7),
                    )
                for db in range(8):
                    nc.tensor.matmul(
                        u_ps[:],
                        lhsT=su_t[db][:],
                        rhs=xc[db][:],
                        start=(db == 0),
                        stop=(db == 7),
                    )
                g_sg = sp.tile([128, CHUNK], f32, tag="tmp", bufs=6, name="g_sg")
                nc.scalar.activation(g_sg[:], g_ps[:], Act.Sigmoid)
                g_s = sp.tile([128, CHUNK], f32, tag="tmp", bufs=6, name="g_s")
                nc.vector.tensor_mul(g_s[:], g_sg[:], g_ps[:])
                s1 = sp.tile([128, CHUNK], f32, tag="tmp", bufs=6, name="s1")
                nc.vector.tensor_mul(s1[:], g_s[:], u_ps[:])
                s2 = sp.tile([128, CHUNK], f32r, tag="s2")
                nc.vector.tensor_mul(s2[:], s1[:], bc_g[:])

                # --- down proj: y^T[D, chunk] = w2^T@hp + sd^T@s2 ---
                yb = dp.tile([D, CHUNK], f32, tag="yb")
                for db in range(8):
                    y_ps = pp.tile([128, CHUNK], f32, tag="y", bufs=2)
                    for fb in range(8):
                        nc.tensor.matmul(
                            y_ps[:],
                            lhsT=w2_t[fb][:, db * 128:(db + 1) * 128],
                            rhs=hp[fb][:],
                            start=(fb == 0),
                            stop=False,
                        )
                    nc.tensor.matmul(
                        y_ps[:],
                        lhsT=sd_t[:, db * 128:(db + 1) * 128],
                        rhs=s2[:],
                        start=False,
                        stop=True,
                    )
                    y_sb = sp.tile([128, CHUNK], f32, tag="y_sb")
                    nc.vector.tensor_copy(y_sb[:], y_ps[:])
                    nc.sync.dma_start(yb[db * 128:(db + 1) * 128, :], y_sb[:])

                # --- reduce across cores; rank r keeps y^T rows [128r,128r+128) ---
                rs = dp.tile([DSH, CHUNK], f32, tag="rs")
                nc.gpsimd.collective_compute(
                    "ReduceScatter",
                    Alu.add,
                    replica_groups=[list(range(NCORES))],
                    ins=[yb.opt()],
                    outs=[rs.opt()],
                )
                nc.sync.dma_start(out[:, tok0:tok0 + CHUNK], rs[:])

    nc.compile()
    return nc


def _get_nc():
    if "nc" not in _CACHE:
        _CACHE["nc"] = _build_nc()
    return _CACHE["nc"]


def make_in_maps(x, router_w, w1, w2, shared_gate_up, shared_down, shared_gate_w):
    xT = np.ascontiguousarray(
        x.reshape(N, D).T.astype(np.float32)
    )
    rw9 = np.ascontiguousarray(
        np.concatenate(
            [router_w.astype(np.float32), shared_gate_w.astype(np.float32)], axis=1
        )
    )
    in_maps = []
    for e in range(NCORES):
        onehot = np.zeros((1, E), np.float32)
        onehot[0, e] = 1.0
        in_maps.append(
            {
                "xT": xT,
                "rw9": rw9,
                "w1": np.ascontiguousarray(w1[e].astype(np.float32)),
                "w2": np.ascontiguousarray(w2[e].astype(np.float32)),
                "sg": np.ascontiguousarray(
                    shared_gate_up[:, e * FSL:(e + 1) * FSL].astype(np.float32)
                ),
                "su": np.ascontiguousarray(
                    shared_gate_up[:, F + e * FSL:F + (e + 1) * FSL].astype(np.float32)
                ),
                "sd": np.ascontiguousarray(
                    shared_down[e * FSL:(e + 1) * FSL, :].astype(np.float32)
                ),
                "esel": onehot,
            }
        )
    return in_maps


def assemble_out(results):
    yT = np.concatenate([results[r]["out"] for r in range(NCORES)], axis=0)
    return np.ascontiguousarray(yT.T).reshape(B, T, D)


def kernel(x, router_w, w1, w2, shared_gate_up, shared_down, shared_gate_w):
    from concourse import bass_utils

    nc = _get_nc()
    in_maps = make_in_maps(
        x, router_w, w1, w2, shared_gate_up, shared_down, shared_gate_w
    )
    res = bass_utils.run_bass_kernel_spmd(
        nc, in_maps, core_ids=list(range(NCORES))
    )
    return assemble_out(res.results)



# revision 5
# speedup vs baseline: 13.5578x; 13.5578x over previous
"""MoE block (8 experts, top-2, shared SwiGLU expert) on 8 TRN2 NeuronCores.

Strategy (expert-parallel):
  - Core e owns expert e: computes c_e(token) * silu(x @ w1[e]) @ w2[e] for ALL
    tokens (dense, combine weight c_e is zero for non-selected tokens).
  - Shared expert is tensor-parallel over its inter dim F: core e owns a 128-wide
    slice of gate/up columns and the matching shared_down rows; the sigmoid token
    gate is folded into the slice contribution before the down matmul.
  - Router (+ shared gate logit as a 9th column) is replicated on every core.
  - Per-core partial y^T [D, N] accumulates routed + shared-slice contributions;
    a ReduceScatter(add) over the 8 cores both reduces and shards the result.
    Core r returns rows [128*r, 128*(r+1)) of the final y^T; the host
    concatenates and transposes.

All matmuls run in float32r (fp32 transpose-mode streaming: 1 cycle/row when the
moving free dim >= 256) with the data shipped pre-transposed (x^T) so every
operand is already in lhsT layout.
"""

import os

import numpy as np

REPEAT = int(os.environ.get("BASS_BENCH_REPEAT", "1"))

D = 1024
F = 1024
E = 8
B, T = 2, 2048
N = B * T          # 4096 tokens
NCORES = 8
CHUNK = 512        # tokens per pipeline chunk
NCHUNK = N // CHUNK
FSL = F // NCORES  # shared-expert inter-dim slice per core
DSH = D // NCORES  # output rows (of y^T) per core after reduce-scatter

_CACHE = {}


def _build_nc():
    import concourse.bacc as bacc
    import concourse.mybir as mybir
    import concourse.tile as tile
    from concourse import masks

    dt = mybir.dt
    f32 = dt.float32
    f32r = dt.float32r
    Act = mybir.ActivationFunctionType
    Alu = mybir.AluOpType
    AX = mybir.AxisListType

    nc = bacc.Bacc(
        "TRN2",
        target_bir_lowering=False,
        debug=False,
        enable_asserts=False,
        num_devices=NCORES,
    )

    xT = nc.dram_tensor("xT", [D, N], f32, kind="ExternalInput").ap()
    rw9 = nc.dram_tensor("rw9", [D, E + 1], f32, kind="ExternalInput").ap()
    w1 = nc.dram_tensor("w1", [D, F], f32, kind="ExternalInput").ap()
    w2 = nc.dram_tensor("w2", [F, D], f32, kind="ExternalInput").ap()
    sg = nc.dram_tensor("sg", [D, FSL], f32, kind="ExternalInput").ap()
    su = nc.dram_tensor("su", [D, FSL], f32, kind="ExternalInput").ap()
    sd = nc.dram_tensor("sd", [FSL, D], f32, kind="ExternalInput").ap()
    esel = nc.dram_tensor("esel", [1, E], f32, kind="ExternalInput").ap()
    out = nc.dram_tensor("out", [DSH, N], f32, kind="ExternalOutput").ap()

    r = lambda ap: ap.bitcast(f32r)

    with tile.TileContext(nc) as tc:
        with (
            tc.tile_pool(name="wp", bufs=1) as wp,
            tc.tile_pool(name="xp", bufs=2) as xp,
            tc.tile_pool(name="sp", bufs=2) as sp,
            tc.tile_pool(name="vp", bufs=2) as vp,
            tc.tile_pool(name="pp", bufs=1, space="PSUM") as pp,
            tc.tile_pool(name="dp", bufs=2, space="DRAM") as dp,
        ):
            # ---- static weights/constants ----
            # f32r matmul operands must be produced as f32r by a compute op,
            # so weights are staged f32 then cast once on DVE.
            w1_t = []
            w2_t = []
            sg_t = []
            su_t = []
            rw_t = []
            with tc.tile_pool(name="stg", bufs=2) as stg:
                def load_r(dst_pool, tag, src_ap, nparts, nfree):
                    st = stg.tile([nparts, nfree], f32, tag="stage", name="st")
                    nc.sync.dma_start(st[:], src_ap)
                    t = dst_pool.tile([nparts, nfree], f32r, tag=tag, name=tag)
                    nc.vector.tensor_copy(t[:], st[:])
                    return t

                for db in range(8):
                    w1_t.append(load_r(wp, f"w1_{db}", w1[db * 128:(db + 1) * 128, :], 128, F))
                for fb in range(8):
                    w2_t.append(load_r(wp, f"w2_{fb}", w2[fb * 128:(fb + 1) * 128, :], 128, D))
                for db in range(8):
                    sg_t.append(load_r(wp, f"sg_{db}", sg[db * 128:(db + 1) * 128, :], 128, FSL))
                    su_t.append(load_r(wp, f"su_{db}", su[db * 128:(db + 1) * 128, :], 128, FSL))
                sd_t = load_r(wp, "sd", sd[:, :], 128, D)
            for db in range(8):
                t = wp.tile([128, E + 1], f32, tag=f"rw_{db}", name="rwt")
                nc.sync.dma_start(t[:], rw9[db * 128:(db + 1) * 128, :])
                rw_t.append(t)
            esel_sb = wp.tile([1, E], f32, tag="esel1")
            nc.sync.dma_start(esel_sb[:], esel[:, :])
            esel_bc = wp.tile([128, E], f32, tag="eselbc")
            nc.gpsimd.partition_broadcast(esel_bc[:], esel_sb[:])
            ident = wp.tile([128, 128], f32, tag="ident")
            masks.make_identity(nc, ident[:])

            # ---- main pipeline over token chunks ----
            for c in range(NCHUNK * REPEAT):
                c = c % NCHUNK
                tok0 = c * CHUNK
                # x^T chunk, 8 partition blocks of [128, CHUNK]
                xcf = []
                xc = []
                for db in range(8):
                    tf_ = xp.tile([128, CHUNK], f32, tag=f"xcf{db}", bufs=1, name="tf_")
                    eng = nc.sync if db % 2 == 0 else nc.scalar
                    eng.dma_start(
                        tf_[:], xT[db * 128:(db + 1) * 128, tok0:tok0 + CHUNK]
                    )
                    xcf.append(tf_)
                    tr_ = xp.tile([128, CHUNK], f32r, tag=f"xc{db}", name="tr_")
                    nc.vector.tensor_copy(tr_[:], tf_[:])
                    xc.append(tr_)

                # --- router + shared gate logit, token-partition layout ---
                cT = vp.tile([1, CHUNK], f32, tag="cT")
                gT = vp.tile([1, CHUNK], f32, tag="gT")
                for s in range(CHUNK // 128):
                    lg_ps = pp.tile([128, E + 1], f32, tag="lg")
                    for db in range(8):
                        nc.tensor.matmul(
                            lg_ps[:],
                            lhsT=xcf[db][:, s * 128:(s + 1) * 128],
                            rhs=rw_t[db][:],
                            start=(db == 0),
                            stop=(db == 7),
                        )
                    lg = vp.tile([128, E], f32, tag="lg_sb")
                    nc.vector.tensor_copy(lg[:], lg_ps[:, 0:E])
                    # softmax numerator + denominator (no max-subtract: |logit|<~6)
                    pe_un = vp.tile([128, E], f32, tag="pe_un")
                    sumx = vp.tile([128, 1], f32, tag="sumx")
                    nc.scalar.activation(
                        pe_un[:], lg_ps[:, 0:E], Act.Exp, accum_out=sumx[:]
                    )
                    rcp = vp.tile([128, 1], f32, tag="rcp")
                    nc.vector.reciprocal(rcp[:], sumx[:])
                    # rank_i = #{j: l_j > l_i}  (strict; top-2 keep rank < 2)
                    cnt = [
                        vp.tile([128, E], f32, tag="cnt0", name="cnt0"),
                        vp.tile([128, E], f32, tag="cnt1", name="cnt1"),
                    ]
                    nc.vector.tensor_scalar(
                        cnt[0][:], lg[:], lg[:, 0:1], None, Alu.is_lt
                    )
                    for j in range(1, E):
                        nc.vector.scalar_tensor_tensor(
                            cnt[j % 2][:],
                            lg[:],
                            lg[:, j:j + 1],
                            cnt[(j + 1) % 2][:],
                            Alu.is_lt,
                            Alu.add,
                        )
                    cfin = cnt[(E - 1) % 2]
                    mask = vp.tile([128, E], f32, tag="mask")
                    nc.vector.tensor_scalar(
                        mask[:], cfin[:], 2.0, None, Alu.is_lt
                    )
                    t1 = vp.tile([128, E], f32, tag="t1")
                    nc.vector.tensor_mul(t1[:], pe_un[:], mask[:])
                    t2 = vp.tile([128, E], f32, tag="t2")
                    nc.vector.tensor_mul(t2[:], t1[:], esel_bc[:])
                    cred = vp.tile([128, 1], f32, tag="cred")
                    nc.vector.reduce_sum(cred[:], t2[:], axis=AX.X)
                    ccol = vp.tile([128, 1], f32, tag="ccol")
                    nc.vector.tensor_scalar_mul(ccol[:], cred[:], rcp[:])
                    sig = vp.tile([128, 1], f32, tag="sig")
                    nc.scalar.activation(sig[:], lg_ps[:, E:E + 1], Act.Sigmoid)
                    # transpose both [128,1] columns into row layout
                    ct_ps = pp.tile([1, 256], f32, tag="ct")
                    nc.tensor.transpose(ct_ps[:, 0:128], ccol[:], ident[:])
                    nc.tensor.transpose(ct_ps[:, 128:256], sig[:], ident[:])
                    nc.vector.tensor_copy(
                        cT[:, s * 128:(s + 1) * 128], ct_ps[:, 0:128]
                    )
                    nc.vector.tensor_copy(
                        gT[:, s * 128:(s + 1) * 128], ct_ps[:, 128:256]
                    )
                bc_c = sp.tile([128, CHUNK], f32, tag="bc_c")
                nc.gpsimd.partition_broadcast(bc_c[:], cT[:])
                bc_g = sp.tile([128, CHUNK], f32, tag="bc_g")
                nc.gpsimd.partition_broadcast(bc_g[:], gT[:])

                # --- expert up-proj + silu + combine scale ---
                hp = []
                for fb in range(8):
                    h_ps = pp.tile([128, CHUNK], f32, tag="h", bufs=2)
                    for db in range(8):
                        nc.tensor.matmul(
                            h_ps[:],
                            lhsT=w1_t[db][:, fb * 128:(fb + 1) * 128],
                            rhs=xc[db][:],
                            start=(db == 0),
                            stop=(db == 7),
                        )
                    h_sg = sp.tile([128, CHUNK], f32, tag="tmp", bufs=6, name="h_sg")
                    nc.scalar.activation(h_sg[:], h_ps[:], Act.Sigmoid)
                    h_s = sp.tile([128, CHUNK], f32, tag="tmp", bufs=6, name="h_s")
                    nc.vector.tensor_mul(h_s[:], h_sg[:], h_ps[:])
                    hpt = sp.tile([128, CHUNK], f32r, tag=f"hp{fb}", name="hpt")
                    nc.vector.tensor_mul(hpt[:], h_s[:], bc_c[:])
                    hp.append(hpt)

                # --- shared expert slice: silu(gate)*up*sigmoid ---
                g_ps = pp.tile([128, CHUNK], f32, tag="g")
                u_ps = pp.tile([128, CHUNK], f32, tag="u")
                for db in range(8):
                    nc.tensor.matmul(
                        g_ps[:],
                        lhsT=sg_t[db][:],
                        rhs=xc[db][:],
                        start=(db == 0),
                        stop=(db == 7),
                    )
                for db in range(8):
                    nc.tensor.matmul(
                        u_ps[:],
                        lhsT=su_t[db][:],
                        rhs=xc[db][:],
                        start=(db == 0),
                        stop=(db == 7),
                    )
                g_sg = sp.tile([128, CHUNK], f32, tag="tmp", bufs=6, name="g_sg")
                nc.scalar.activation(g_sg[:], g_ps[:], Act.Sigmoid)
                g_s = sp.tile([128, CHUNK], f32, tag="tmp", bufs=6, name="g_s")
                nc.vector.tensor_mul(g_s[:], g_sg[:], g_ps[:])
                s1 = sp.tile([128, CHUNK], f32, tag="tmp", bufs=6, name="s1")
                nc.vector.tensor_mul(s1[:], g_s[:], u_ps[:])
                s2 = sp.tile([128, CHUNK], f32r, tag="s2")
                nc.vector.tensor_mul(s2[:], s1[:], bc_g[:])

                # --- down proj: y^T[D, chunk] = w2^T@hp + sd^T@s2 ---
                yb = dp.tile([D, CHUNK], f32, tag="yb")
                for db in range(8):
                    y_ps = pp.tile([128, CHUNK], f32, tag="y", bufs=2)
                    for fb in range(8):
                        nc.tensor.matmul(
                            y_ps[:],
                            lhsT=w2_t[fb][:, db * 128:(db + 1) * 128],
                            rhs=hp[fb][:],
                            start=(fb == 0),
                            stop=False,
                        )
                    nc.tensor.matmul(
                        y_ps[:],
                        lhsT=sd_t[:, db * 128:(db + 1) * 128],
                        rhs=s2[:],
                        start=False,
                        stop=True,
                    )
                    y_sb = sp.tile([128, CHUNK], f32, tag="y_sb")
                    nc.vector.tensor_copy(y_sb[:], y_ps[:])
                    nc.sync.dma_start(yb[db * 128:(db + 1) * 128, :], y_sb[:])

                # --- reduce across cores; rank r keeps y^T rows [128r,128r+128) ---
                rs = dp.tile([DSH, CHUNK], f32, tag="rs")
                nc.gpsimd.collective_compute(
                    "ReduceScatter",
                    Alu.add,
                    replica_groups=[list(range(NCORES))],
                    ins=[yb.opt()],
                    outs=[rs.opt()],
                )
                nc.sync.dma_start(out[:, tok0:tok0 + CHUNK], rs[:])

    nc.compile()
    return nc


def _get_nc():
    if "nc" not in _CACHE:
        _CACHE["nc"] = _build_nc()
    return _CACHE["nc"]


def make_in_maps(x, router_w, w1, w2, shared_gate_up, shared_down, shared_gate_w):
    xT = np.ascontiguousarray(
        x.reshape(N, D).T.astype(np.float32)
    )
    rw9 = np.ascontiguousarray(
        np.concatenate(
            [router_w.astype(np.float32), shared_gate_w.astype(np.float32)], axis=1
        )
    )
    in_maps = []
    for e in range(NCORES):
        onehot = np.zeros((1, E), np.float32)
        onehot[0, e] = 1.0
        in_maps.append(
            {
                "xT": xT,
                "rw9": rw9,
                "w1": np.ascontiguousarray(w1[e].astype(np.float32)),
                "w2": np.ascontiguousarray(w2[e].astype(np.float32)),
                "sg": np.ascontiguousarray(
                    shared_gate_up[:, e * FSL:(e + 1) * FSL].astype(np.float32)
                ),
                "su": np.ascontiguousarray(
                    shared_gate_up[:, F + e * FSL:F + (e + 1) * FSL].astype(np.float32)
                ),
                "sd": np.ascontiguousarray(
                    shared_down[e * FSL:(e + 1) * FSL, :].astype(np.float32)
                ),
                "esel": onehot,
            }
        )
    return in_maps


def assemble_out(results):
    yT = np.concatenate([results[r]["out"] for r in range(NCORES)], axis=0)
    return np.ascontiguousarray(yT.T).reshape(B, T, D)


def kernel(x, router_w, w1, w2, shared_gate_up, shared_down, shared_gate_w):
    from concourse import bass_utils

    nc = _get_nc()
    in_maps = make_in_maps(
        x, router_w, w1, w2, shared_gate_up, shared_down, shared_gate_w
    )
    res = bass_utils.run_bass_kernel_spmd(
        nc, in_maps, core_ids=list(range(NCORES))
    )
    return assemble_out(res.results)

